# revision 1
# baseline (speedup 1.0000x reference)
"""ChannelAttention1D (SE-MLP over frame means + top-k frame gather) on 8 trn2 cores.

Math (per sample b of B=4096):
    pooled[f] = mean_d x[b, f, d]                    f in [0, 64)
    h = relu(pooled @ w1.T + b1)                     [16]
    logits = h @ w2.T + b2                           [64]  (sigmoid is monotonic -> skipped)
    idx = top_k(logits, 8).indices
    out[b, k, :] = x[b, idx[k], :]

Sharding: pure data-parallel over batch, 512 samples per core; SE weights replicated.

Per-core kernel (all shapes hardcoded), sample-major layout throughout — one
sample per SBUF partition, so the whole compute chain lives on the DVE and no
cross-engine (PE/ACT) hops sit on the critical path. An earlier PE/ACT-based
variant (f-major pooled, PE matmuls + transpose) consistently measured slower:
the Tile scheduler batches same-kind PE ops across groups, which serialized
every group's topk/gather behind the last group's reduce (a ~60 us dependency
tail in the cost-model timeline, and a large same-session A/B gap on HW).

  Per group of 128 samples:
    - two 8MB HWDGE loads x[g*128:(g+1)*128, 32-frame slice, :] -> [128, 32,
      512] tiles (64KB contiguous per partition)
    - two-stage DVE reduce over D (inner 32, then 16) -> pooled sums [128, 64]
    - SE-MLP on DVE via broadcast tensor_tensor multiplies against
      host-replicated weight rows + segmented tensor_reduce:
        h = relu(sum_f pooled*w1bc + b1), logits = sum_r h*w2bc + b2
      (the 1/512 mean scale is folded into w1bc on the host; exact, power of 2)
    - top-8 via the DVE MAX + MAX_INDEX instructions (K=8 = HW width)
    - global row ids grow[p, k] = (g*128+p)*64 + idx, computed in f32 (exact)
    - 8 indirect SWDGE gathers (one per rank k, one offset per partition) pull
      the selected 2KB frame rows from HBM into gt [128, 8, 512]
    - one contiguous 2MB HWDGE store gt -> out[g*128:(g+1)*128]

  DMA split: x-loads on the sync (SP) HWDGE ring, output stores on the scalar
  (ACT) ring, constants + indirect gathers on SWDGE (gpsimd).

  Built on Bacc (not raw Bass) so nc.compile() runs generate_event_semaphores:
  this walrus permits at most one sync wait per instruction, and that pass
  splits multi-waits into EventSemaphore ops.
"""

import sys

sys.path.insert(0, "/opt/trn_rl_repo")

import numpy as np

B, F, D = 4096, 64, 512
K = 8
NCORES = 8
BL = B // NCORES  # 512 samples per core
GROUPS = BL // 128  # 4
NLOAD = 2  # x-load DMAs per group
FPL = F // NLOAD  # frames per load = 32
R = 16  # SE bottleneck width

_cache = {}


def _build_nc():
    import concourse.bass as bass
    import concourse.bacc as bacc
    import concourse.mybir as mybir
    import concourse.tile as tile
    from contextlib import ExitStack

    f32 = mybir.dt.float32
    u32 = mybir.dt.uint32
    X = mybir.AxisListType.X
    ADD = mybir.AluOpType.add
    MULT = mybir.AluOpType.mult

    nc = bacc.Bacc(
        "TRN2", target_bir_lowering=False, debug=False, num_devices=NCORES
    )
    x = nc.declare_dram_parameter("x", [BL, F, D], f32, isOutput=False)
    w1bc = nc.declare_dram_parameter("w1bc", [128, R * F], f32, isOutput=False)
    w2bc = nc.declare_dram_parameter("w2bc", [128, F * R], f32, isOutput=False)
    b1b = nc.declare_dram_parameter("b1b", [128, R], f32, isOutput=False)
    b2b = nc.declare_dram_parameter("b2b", [128, F], f32, isOutput=False)
    offc = nc.declare_dram_parameter("offc", [128, GROUPS], f32, isOutput=False)
    out = nc.declare_dram_parameter("out", [BL, K, D], f32, isOutput=True)

    x_ap = x[:]
    x_flat = x_ap.rearrange("b f d -> (b f) d")  # [BL*64, 512]
    out_ap = out[:]

    def bcast_mid(ap, n):
        # [P, A] -> [P, n, A] with a stride-0 broadcast middle dim
        return bass.AP(
            tensor=ap.tensor, offset=ap.offset, ap=[ap.ap[0], [0, n], *ap.ap[1:]]
        )

    with ExitStack() as ctx:
        tc = ctx.enter_context(tile.TileContext(nc))
        consts = ctx.enter_context(tc.tile_pool(name="consts", bufs=1))
        xpool = ctx.enter_context(tc.tile_pool(name="xpool", bufs=2))
        small = ctx.enter_context(tc.tile_pool(name="small", bufs=2))
        gpool = ctx.enter_context(tc.tile_pool(name="gpool", bufs=2))

        w1bc_sb = consts.tile([128, R * F], f32)
        nc.gpsimd.dma_start(out=w1bc_sb[:], in_=w1bc[:])
        w2bc_sb = consts.tile([128, F * R], f32)
        nc.gpsimd.dma_start(out=w2bc_sb[:], in_=w2bc[:])
        b1b_sb = consts.tile([128, R], f32)
        nc.gpsimd.dma_start(out=b1b_sb[:], in_=b1b[:])
        b2b_sb = consts.tile([128, F], f32)
        nc.gpsimd.dma_start(out=b2b_sb[:], in_=b2b[:])
        offc_sb = consts.tile([128, GROUPS], f32)
        nc.gpsimd.dma_start(out=offc_sb[:], in_=offc[:])

        for g in range(GROUPS):
            s0 = g * 128
            # ---- pooled sums [sample, frame] ----
            pooled = small.tile([128, F], f32, tag="pooled")
            for i in range(NLOAD):
                xt = xpool.tile([128, FPL, D], f32, tag="xt")
                nc.sync.dma_start(
                    out=xt[:],
                    in_=x_ap[s0 : s0 + 128, i * FPL : (i + 1) * FPL, :],
                )
                part = small.tile([128, FPL, 16], f32, tag="part")
                nc.vector.tensor_reduce(
                    out=part[:],
                    in_=xt[:].rearrange("p c (s t) -> p c s t", t=32),
                    axis=X,
                    op=ADD,
                )
                nc.vector.tensor_reduce(
                    out=pooled[:, i * FPL : (i + 1) * FPL], in_=part[:], axis=X, op=ADD
                )

            # ---- SE-MLP, entirely on DVE ----
            # h[p, r] = sum_f pooled[p, f] * w1bc[p, r*64+f]  (w1bc pre-scaled)
            tmp1 = small.tile([128, R, F], f32, tag="tmp1")
            nc.vector.tensor_tensor(
                out=tmp1[:],
                in0=bcast_mid(pooled[:], R),
                in1=w1bc_sb[:].rearrange("p (r f) -> p r f", f=F),
                op=MULT,
            )
            hr = small.tile([128, R], f32, tag="hr")
            nc.vector.tensor_reduce(out=hr[:], in_=tmp1[:], axis=X, op=ADD)
            nc.vector.tensor_add(out=hr[:], in0=hr[:], in1=b1b_sb[:])
            nc.vector.tensor_scalar_max(hr[:], hr[:], 0.0)

            # logits[p, f] = sum_r h[p, r] * w2bc[p, f*16+r]
            tmp2 = small.tile([128, F, R], f32, tag="tmp2")
            nc.vector.tensor_tensor(
                out=tmp2[:],
                in0=bcast_mid(hr[:], F),
                in1=w2bc_sb[:].rearrange("p (f r) -> p f r", r=R),
                op=MULT,
            )
            lg = small.tile([128, F], f32, tag="lg")
            nc.vector.tensor_reduce(out=lg[:], in_=tmp2[:], axis=X, op=ADD)
            nc.vector.tensor_add(out=lg[:], in0=lg[:], in1=b2b_sb[:])

            # ---- top-8 ----
            mx8 = small.tile([128, K], f32, tag="mx8")
            idx8 = small.tile([128, K], u32, tag="idx8")
            nc.vector.max(out=mx8[:], in_=lg[:])
            nc.vector.max_index(out=idx8[:], in_max=mx8[:], in_values=lg[:])

            # global x_flat row id = (g*128 + p)*64 + idx; f32 math is exact
            # for values < 2^24 and tensor_scalar AP operands must be f32
            idx8f = small.tile([128, K], f32, tag="idx8f")
            nc.vector.tensor_copy(out=idx8f[:], in_=idx8[:])
            nc.vector.tensor_scalar(
                out=idx8f[:],
                in0=idx8f[:],
                scalar1=offc_sb[:, g : g + 1],
                scalar2=None,
                op0=ADD,
            )
            grow = small.tile([128, K], u32, tag="grow")
            nc.vector.tensor_copy(out=grow[:], in_=idx8f[:])

            # ---- gather + store ----
            gt = gpool.tile([128, K, D], f32, tag="gt")
            for k in range(K):
                nc.gpsimd.indirect_dma_start(
                    out=gt[:, k, :],
                    out_offset=None,
                    in_=x_flat,
                    in_offset=bass.IndirectOffsetOnAxis(ap=grow[:, k : k + 1], axis=0),
                )
            nc.scalar.dma_start(out=out_ap[s0 : s0 + 128, :, :], in_=gt[:])

    nc.compile()
    return nc


def _consts(w1, b1, w2, b2):
    w1s = (np.asarray(w1, np.float32) / 512.0).reshape(1, -1)  # [1, 16*64], r-major
    w2f = np.asarray(w2, np.float32).reshape(1, -1)  # [1, 64*16], f-major
    w1bc = np.ascontiguousarray(np.tile(w1s, (128, 1)))
    w2bc = np.ascontiguousarray(np.tile(w2f, (128, 1)))
    b1b = np.tile(np.asarray(b1, np.float32)[None, :], (128, 1))
    b2b = np.tile(np.asarray(b2, np.float32)[None, :], (128, 1))
    p = np.arange(128)
    offc = (
        (p[:, None] + np.arange(GROUPS)[None, :] * 128) * 64
    ).astype(np.float32)
    return w1bc, w2bc, b1b, b2b, offc


def make_in_maps(x, w1, b1, w2, b2):
    x = np.asarray(x)
    w1bc, w2bc, b1b, b2b, offc = _consts(
        np.asarray(w1), np.asarray(b1), np.asarray(w2), np.asarray(b2)
    )
    in_maps = []
    for i in range(NCORES):
        in_maps.append(
            {
                "x": np.ascontiguousarray(x[i * BL : (i + 1) * BL]),
                "w1bc": w1bc,
                "w2bc": w2bc,
                "b1b": b1b,
                "b2b": b2b,
                "offc": offc,
            }
        )
    return in_maps


def build_nc():
    if "nc" not in _cache:
        _cache["nc"] = _build_nc()
    return _cache["nc"]


def _fast_call(in_maps):
    """Cached jit of the 8-core NEFF (same construction run_bass_kernel_spmd
    lowers to under axon); repeat kernel() calls skip re-tracing/re-jitting."""
    import jax
    from jax.sharding import Mesh, PartitionSpec
    from jax.experimental.shard_map import shard_map
    from concourse import bass2jax, mybir

    if "fast" not in _cache:
        nc = build_nc()
        bass2jax.install_neuronx_cc_hook()
        in_names, out_names, out_avals = [], [], []
        for alloc in nc.m.functions[0].allocations:
            if not isinstance(alloc, mybir.MemoryLocationSet):
                continue
            name = alloc.memorylocations[0].name
            if alloc.kind == "ExternalInput":
                in_names.append(name)
            elif alloc.kind == "ExternalOutput":
                out_names.append(name)
                out_avals.append(
                    jax.core.ShapedArray(
                        tuple(alloc.tensor_shape), mybir.dt.np(alloc.dtype)
                    )
                )
        all_in = list(in_names) + list(out_names)

        def _body(*args):
            return tuple(
                bass2jax._bass_exec_p.bind(
                    *args,
                    out_avals=tuple(out_avals),
                    in_names=tuple(all_in),
                    out_names=tuple(out_names),
                    lowering_input_output_aliases=(),
                    sim_require_finite=True,
                    sim_require_nnan=True,
                    nc=nc,
                )
            )

        mesh = Mesh(np.asarray(jax.devices()[:NCORES]), ("core",))
        nin = len(in_names) + len(out_names)
        f = jax.jit(
            shard_map(
                _body,
                mesh=mesh,
                in_specs=(PartitionSpec("core"),) * nin,
                out_specs=(PartitionSpec("core"),) * len(out_names),
                check_rep=False,
            ),
            keep_unused=True,
        )
        _cache["fast"] = (f, in_names, out_names, out_avals)
    f, in_names, out_names, out_avals = _cache["fast"]
    concat_in = [
        np.concatenate([np.asarray(m[nm]) for m in in_maps], axis=0)
        for nm in in_names
    ]
    concat_zero = [
        np.zeros((NCORES * a.shape[0], *a.shape[1:]), a.dtype) for a in out_avals
    ]
    outs = f(*concat_in, *concat_zero)
    oi = out_names.index("out")
    return np.asarray(outs[oi]).reshape(NCORES * BL, K, D)


def kernel(x, w1, b1, w2, b2):
    import os

    # the NTFF trace hook (antenv.axon_hooks) doesn't exist in this container;
    # make sure an inherited BASS_TRACE can't route us onto that path
    os.environ["BASS_NEVER_TRACE"] = "1"
    in_maps = make_in_maps(x, w1, b1, w2, b2)
    try:
        return _fast_call(in_maps)
    except Exception:
        from concourse.bass_utils import run_bass_kernel_spmd

        res = run_bass_kernel_spmd(build_nc(), in_maps, list(range(NCORES)))
        return np.concatenate([r["out"] for r in res.results], axis=0)



# revision 12
# speedup vs baseline: 2.2824x; 2.2824x over previous
"""ChannelAttention1D (SE-MLP over frame means + top-k frame gather) on 8 trn2 cores.

Math (per sample b of B=4096):
    pooled[f] = mean_d x[b, f, d]                    f in [0, 64)
    h = relu(pooled @ w1.T + b1)                     [16]
    logits = h @ w2.T + b2                           [64]  (sigmoid is monotonic -> skipped)
    idx = top_k(logits, 8).indices
    out[b, k, :] = x[b, idx[k], :]

Sharding: pure data-parallel over batch, 512 samples per core; SE weights replicated.

Per-core kernel (all shapes hardcoded), sample-major layout - one sample per
SBUF partition. v1.5: same DMA structure as the 171us v1 baseline (HWDGE
x-loads, 8x [128,1]-offset SWDGE gathers from HBM, contiguous store - the
HW SWDGE indirect path only supports ONE offset per partition per DMA, so
SBUF-source scatters with [128,N] offset tables are not expressible), plus
the frame-sum reduction split across engines:

  - ACT sums frames 0..31 (one activation(Copy, accum_out) per frame,
    sequential-512 accumulate) from the first 32-frame tile.
  - DVE keeps the exact v1 two-stage tree (32-chunk tensor_reduce then
    16-partial reduce) for frames 32..63, plus SE-MLP / top-8 / row-id math.
  DVE busy drops ~163us -> ~105us, under the ~166us DMA roofline (80 MiB
  of SDMA traffic/core), and x tiles are released sooner for prefetch.

  Per group of 128 samples:
    - two 8MB HWDGE loads x[g*128:(g+1)*128, 32-frame slice, :] -> [128, 32,
      512] tiles (64KB contiguous per partition)
    - tile 0: 32x ACT activation(Copy, accum_out=pooled[:, f])
    - tile 1: two-stage DVE reduce (inner 32, then 16) -> pooled[:, 32:64]
    - SE-MLP on DVE via broadcast tensor_tensor multiplies against
      host-replicated weight rows + segmented tensor_reduce:
        h = relu(sum_f pooled*w1bc + b1), logits = sum_r h*w2bc + b2
      (the 1/512 mean scale is folded into w1bc on the host; exact, power of 2)
    - top-8 via the DVE MAX + MAX_INDEX instructions (K=8 = HW width)
    - global row ids grow[p, k] = (g*128+p)*64 + idx, computed in f32 (exact)
    - 8 indirect SWDGE gathers (one per rank k, one offset per partition) pull
      the selected 2KB frame rows from HBM into gt [128, 8, 512]
    - one contiguous 2MB HWDGE store gt -> out[g*128:(g+1)*128]

  DMA split: x-loads on the sync (SP) HWDGE ring; constants, indirect
  gathers and output stores on SWDGE (gpsimd) - the scalar HWDGE ring would
  head-of-line-block the ACT compute stream, and SP would block loads.

  Built on Bacc (not raw Bass) so nc.compile() runs generate_event_semaphores:
  this walrus permits at most one sync wait per instruction, and that pass
  splits multi-waits into EventSemaphore ops.
"""

import sys

sys.path.insert(0, "/opt/trn_rl_repo")

import numpy as np

B, F, D = 4096, 64, 512
K = 8
NCORES = 8
BL = B // NCORES  # 512 samples per core
GROUPS = BL // 128  # 4
NLOAD = 2  # x-load DMAs per group
FPL = F // NLOAD  # frames per load = 32
R = 16  # SE bottleneck width

_cache = {}


def _build_nc():
    import concourse.bass as bass
    import concourse.bacc as bacc
    import concourse.mybir as mybir
    import concourse.tile as tile
    from contextlib import ExitStack

    f32 = mybir.dt.float32
    u32 = mybir.dt.uint32
    X = mybir.AxisListType.X
    ADD = mybir.AluOpType.add
    MULT = mybir.AluOpType.mult
    COPY = mybir.ActivationFunctionType.Copy

    nc = bacc.Bacc(
        "TRN2", target_bir_lowering=False, debug=False, num_devices=NCORES
    )
    x = nc.declare_dram_parameter("x", [BL, F, D], f32, isOutput=False)
    w1bc = nc.declare_dram_parameter("w1bc", [128, R * F], f32, isOutput=False)
    w2bc = nc.declare_dram_parameter("w2bc", [128, F * R], f32, isOutput=False)
    b1b = nc.declare_dram_parameter("b1b", [128, R], f32, isOutput=False)
    b2b = nc.declare_dram_parameter("b2b", [128, F], f32, isOutput=False)
    offc = nc.declare_dram_parameter("offc", [128, GROUPS], f32, isOutput=False)
    out = nc.declare_dram_parameter("out", [BL, K, D], f32, isOutput=True)

    x_ap = x[:]
    x_flat = x_ap.rearrange("b f d -> (b f) d")  # [BL*64, 512]
    out_ap = out[:]

    def bcast_mid(ap, n):
        # [P, A] -> [P, n, A] with a stride-0 broadcast middle dim
        return bass.AP(
            tensor=ap.tensor, offset=ap.offset, ap=[ap.ap[0], [0, n], *ap.ap[1:]]
        )

    with ExitStack() as ctx:
        tc = ctx.enter_context(tile.TileContext(nc))
        consts = ctx.enter_context(tc.tile_pool(name="consts", bufs=1))
        xpool = ctx.enter_context(tc.tile_pool(name="xpool", bufs=2))
        small = ctx.enter_context(tc.tile_pool(name="small", bufs=2))
        gpool = ctx.enter_context(tc.tile_pool(name="gpool", bufs=2))

        w1bc_sb = consts.tile([128, R * F], f32)
        nc.gpsimd.dma_start(out=w1bc_sb[:], in_=w1bc[:])
        w2bc_sb = consts.tile([128, F * R], f32)
        nc.gpsimd.dma_start(out=w2bc_sb[:], in_=w2bc[:])
        b1b_sb = consts.tile([128, R], f32)
        nc.gpsimd.dma_start(out=b1b_sb[:], in_=b1b[:])
        b2b_sb = consts.tile([128, F], f32)
        nc.gpsimd.dma_start(out=b2b_sb[:], in_=b2b[:])
        offc_sb = consts.tile([128, GROUPS], f32)
        nc.gpsimd.dma_start(out=offc_sb[:], in_=offc[:])
        dummy = consts.tile([128, D], f32)  # ACT elementwise out sink

        for g in range(GROUPS):
            s0 = g * 128
            pooled = small.tile([128, F], f32, tag="pooled")
            for i in range(NLOAD):
                xt = xpool.tile([128, FPL, D], f32, tag="xt")
                nc.sync.dma_start(
                    out=xt[:],
                    in_=x_ap[s0 : s0 + 128, i * FPL : (i + 1) * FPL, :],
                )
                if i == 0:
                    # ---- ACT frame sums (sequential-512 accumulate) ----
                    for fl in range(FPL):
                        nc.scalar.activation(
                            out=dummy[:],
                            in_=xt[:, fl, :],
                            func=COPY,
                            accum_out=pooled[:, fl : fl + 1],
                        )
                else:
                    # Tile 1 is split: DVE two-stage tree on frames 32..47,
                    # ACT sequential sums on 48..63 - both run right after
                    # the load, halving the last-group drain tail.
                    part = small.tile([128, FPL // 2, 16], f32, tag="part")
                    nc.vector.tensor_reduce(
                        out=part[:],
                        in_=xt[:, 0 : FPL // 2, :].rearrange(
                            "p c (s t) -> p c s t", t=32
                        ),
                        axis=X,
                        op=ADD,
                    )
                    nc.vector.tensor_reduce(
                        out=pooled[:, FPL : FPL + FPL // 2],
                        in_=part[:],
                        axis=X,
                        op=ADD,
                    )
                    for fl in range(FPL // 2, FPL):
                        nc.scalar.activation(
                            out=dummy[:],
                            in_=xt[:, fl, :],
                            func=COPY,
                            accum_out=pooled[:, FPL + fl : FPL + fl + 1],
                        )

            # ---- SE-MLP, entirely on DVE ----
            # h[p, r] = sum_f pooled[p, f] * w1bc[p, r*64+f]  (w1bc pre-scaled)
            tmp1 = small.tile([128, R, F], f32, tag="tmp1")
            nc.vector.tensor_tensor(
                out=tmp1[:],
                in0=bcast_mid(pooled[:], R),
                in1=w1bc_sb[:].rearrange("p (r f) -> p r f", f=F),
                op=MULT,
            )
            hr = small.tile([128, R], f32, tag="hr")
            nc.vector.tensor_reduce(out=hr[:], in_=tmp1[:], axis=X, op=ADD)
            nc.vector.tensor_add(out=hr[:], in0=hr[:], in1=b1b_sb[:])
            nc.vector.tensor_scalar_max(hr[:], hr[:], 0.0)

            # logits[p, f] = sum_r h[p, r] * w2bc[p, f*16+r]
            tmp2 = small.tile([128, F, R], f32, tag="tmp2")
            nc.vector.tensor_tensor(
                out=tmp2[:],
                in0=bcast_mid(hr[:], F),
                in1=w2bc_sb[:].rearrange("p (f r) -> p f r", r=R),
                op=MULT,
            )
            lg = small.tile([128, F], f32, tag="lg")
            nc.vector.tensor_reduce(out=lg[:], in_=tmp2[:], axis=X, op=ADD)
            nc.vector.tensor_add(out=lg[:], in0=lg[:], in1=b2b_sb[:])

            # ---- top-8 ----
            mx8 = small.tile([128, K], f32, tag="mx8")
            idx8 = small.tile([128, K], u32, tag="idx8")
            nc.vector.max(out=mx8[:], in_=lg[:])
            nc.vector.max_index(out=idx8[:], in_max=mx8[:], in_values=lg[:])

            # global x_flat row id = (g*128 + p)*64 + idx; f32 math is exact
            # for values < 2^24 and tensor_scalar AP operands must be f32
            idx8f = small.tile([128, K], f32, tag="idx8f")
            nc.vector.tensor_copy(out=idx8f[:], in_=idx8[:])
            nc.vector.tensor_scalar(
                out=idx8f[:],
                in0=idx8f[:],
                scalar1=offc_sb[:, g : g + 1],
                scalar2=None,
                op0=ADD,
            )
            grow = small.tile([128, K], u32, tag="grow")
            nc.vector.tensor_copy(out=grow[:], in_=idx8f[:])

            # ---- gather + store ----
            gt = gpool.tile([128, K, D], f32, tag="gt")
            for k in range(K):
                nc.gpsimd.indirect_dma_start(
                    out=gt[:, k, :],
                    out_offset=None,
                    in_=x_flat,
                    in_offset=bass.IndirectOffsetOnAxis(ap=grow[:, k : k + 1], axis=0),
                )
            nc.gpsimd.dma_start(out=out_ap[s0 : s0 + 128, :, :], in_=gt[:])

    nc.compile()
    return nc


def _consts(w1, b1, w2, b2):
    w1s = (np.asarray(w1, np.float32) / 512.0).reshape(1, -1)  # [1, 16*64], r-major
    w2f = np.asarray(w2, np.float32).reshape(1, -1)  # [1, 64*16], f-major
    w1bc = np.ascontiguousarray(np.tile(w1s, (128, 1)))
    w2bc = np.ascontiguousarray(np.tile(w2f, (128, 1)))
    b1b = np.tile(np.asarray(b1, np.float32)[None, :], (128, 1))
    b2b = np.tile(np.asarray(b2, np.float32)[None, :], (128, 1))
    p = np.arange(128)
    offc = (
        (p[:, None] + np.arange(GROUPS)[None, :] * 128) * 64
    ).astype(np.float32)
    return w1bc, w2bc, b1b, b2b, offc


def make_in_maps(x, w1, b1, w2, b2):
    x = np.asarray(x)
    w1bc, w2bc, b1b, b2b, offc = _consts(
        np.asarray(w1), np.asarray(b1), np.asarray(w2), np.asarray(b2)
    )
    in_maps = []
    for i in range(NCORES):
        in_maps.append(
            {
                "x": np.ascontiguousarray(x[i * BL : (i + 1) * BL]),
                "w1bc": w1bc,
                "w2bc": w2bc,
                "b1b": b1b,
                "b2b": b2b,
                "offc": offc,
            }
        )
    return in_maps


def build_nc():
    if "nc" not in _cache:
        _cache["nc"] = _build_nc()
    return _cache["nc"]


def _assemble(by_name):
    return by_name["out"].reshape(NCORES * BL, K, D)


def _fast_call(in_maps):
    """Cached jit of the 8-core NEFF (same construction run_bass_kernel_spmd
    lowers to under axon); repeat kernel() calls skip re-tracing/re-jitting."""
    import jax
    from jax.sharding import Mesh, PartitionSpec
    from jax.experimental.shard_map import shard_map
    from concourse import bass2jax, mybir

    if "fast" not in _cache:
        nc = build_nc()
        bass2jax.install_neuronx_cc_hook()
        partition_name = (
            nc.partition_id_tensor.name if nc.partition_id_tensor else None
        )
        in_names, out_names, out_avals = [], [], []
        for alloc in nc.m.functions[0].allocations:
            if not isinstance(alloc, mybir.MemoryLocationSet):
                continue
            name = alloc.memorylocations[0].name
            if alloc.kind == "ExternalInput":
                if name != partition_name:
                    in_names.append(name)
            elif alloc.kind == "ExternalOutput":
                out_names.append(name)
                out_avals.append(
                    jax.core.ShapedArray(
                        tuple(alloc.tensor_shape), mybir.dt.np(alloc.dtype)
                    )
                )
        all_in = list(in_names) + list(out_names)
        if partition_name:
            all_in.append(partition_name)

        def _body(*args):
            operands = list(args)
            if partition_name:
                operands.append(bass2jax.partition_id_tensor())
            return tuple(
                bass2jax._bass_exec_p.bind(
                    *operands,
                    out_avals=tuple(out_avals),
                    in_names=tuple(all_in),
                    out_names=tuple(out_names),
                    lowering_input_output_aliases=(),
                    sim_require_finite=True,
                    sim_require_nnan=True,
                    nc=nc,
                )
            )

        mesh = Mesh(np.asarray(jax.devices()[:NCORES]), ("core",))
        nin = len(in_names) + len(out_names)
        f = jax.jit(
            shard_map(
                _body,
                mesh=mesh,
                in_specs=(PartitionSpec("core"),) * nin,
                out_specs=(PartitionSpec("core"),) * len(out_names),
                check_rep=False,
            ),
            keep_unused=True,
        )
        _cache["fast"] = (f, in_names, out_names, out_avals)
    f, in_names, out_names, out_avals = _cache["fast"]
    concat_in = [
        np.concatenate([np.asarray(m[nm]) for m in in_maps], axis=0)
        for nm in in_names
    ]
    concat_zero = [
        np.zeros((NCORES * a.shape[0], *a.shape[1:]), a.dtype) for a in out_avals
    ]
    outs = f(*concat_in, *concat_zero)
    oi = out_names.index("out")
    return np.asarray(outs[oi]).reshape(NCORES * BL, K, D)


def kernel(x, w1, b1, w2, b2):
    import os

    # the NTFF trace hook (antenv.axon_hooks) doesn't exist in this container;
    # make sure an inherited BASS_TRACE can't route us onto that path
    os.environ["BASS_NEVER_TRACE"] = "1"
    in_maps = make_in_maps(x, w1, b1, w2, b2)
    try:
        return _fast_call(in_maps)
    except Exception:
        from concourse.bass_utils import run_bass_kernel_spmd

        res = run_bass_kernel_spmd(build_nc(), in_maps, list(range(NCORES)))
        return np.concatenate([r["out"] for r in res.results], axis=0)


# revision 13
# speedup vs baseline: 2.7849x; 1.2202x over previous
"""ChannelAttention1D (SE-MLP over frame means + top-k frame gather) on 8 trn2 cores.

Math (per sample b of B=4096):
    pooled[f] = mean_d x[b, f, d]                    f in [0, 64)
    h = relu(pooled @ w1.T + b1)                     [16]
    logits = h @ w2.T + b2                           [64]  (sigmoid is monotonic -> skipped)
    idx = top_k(logits, 8).indices
    out[b, k, :] = x[b, idx[k], :]

Sharding: pure data-parallel over batch, 512 samples per core; SE weights replicated.

Per-core kernel (all shapes hardcoded), sample-major layout - one sample per
SBUF partition. v1.5: same DMA structure as the 171us v1 baseline (HWDGE
x-loads, 8x [128,1]-offset SWDGE gathers from HBM, contiguous store - the
HW SWDGE indirect path only supports ONE offset per partition per DMA, so
SBUF-source scatters with [128,N] offset tables are not expressible), plus
the frame-sum reduction split across engines:

  - ACT sums frames 0..31 (one activation(Copy, accum_out) per frame,
    sequential-512 accumulate) from the first 32-frame tile.
  - DVE keeps the exact v1 two-stage tree (32-chunk tensor_reduce then
    16-partial reduce) for frames 32..63, plus SE-MLP / top-8 / row-id math.
  DVE busy drops ~163us -> ~105us, under the ~166us DMA roofline (80 MiB
  of SDMA traffic/core), and x tiles are released sooner for prefetch.

  Per group of 128 samples:
    - two 8MB HWDGE loads x[g*128:(g+1)*128, 32-frame slice, :] -> [128, 32,
      512] tiles (64KB contiguous per partition)
    - tile 0: 32x ACT activation(Copy, accum_out=pooled[:, f])
    - tile 1: two-stage DVE reduce (inner 32, then 16) -> pooled[:, 32:64]
    - SE-MLP on DVE via broadcast tensor_tensor multiplies against
      host-replicated weight rows + segmented tensor_reduce:
        h = relu(sum_f pooled*w1bc + b1), logits = sum_r h*w2bc + b2
      (the 1/512 mean scale is folded into w1bc on the host; exact, power of 2)
    - top-8 via the DVE MAX + MAX_INDEX instructions (K=8 = HW width)
    - global row ids grow[p, k] = (g*128+p)*64 + idx, computed in f32 (exact)
    - 8 indirect SWDGE gathers (one per rank k, one offset per partition) pull
      the selected 2KB frame rows from HBM into gt [128, 8, 512]
    - one contiguous 2MB HWDGE store gt -> out[g*128:(g+1)*128]

  DMA split: x-loads on the sync (SP) HWDGE ring; constants, indirect
  gathers and output stores on SWDGE (gpsimd) - the scalar HWDGE ring would
  head-of-line-block the ACT compute stream, and SP would block loads.

  Built on Bacc (not raw Bass) so nc.compile() runs generate_event_semaphores:
  this walrus permits at most one sync wait per instruction, and that pass
  splits multi-waits into EventSemaphore ops.
"""

import sys

sys.path.insert(0, "/opt/trn_rl_repo")

import numpy as np

B, F, D = 4096, 64, 512
K = 8
NCORES = 8
BL = B // NCORES  # 512 samples per core
GROUPS = BL // 128  # 4
NLOAD = 2  # x-load DMAs per group
FPL = F // NLOAD  # frames per load = 32
R = 16  # SE bottleneck width

_cache = {}


def _build_nc():
    import concourse.bass as bass
    import concourse.bacc as bacc
    import concourse.mybir as mybir
    import concourse.tile as tile
    from contextlib import ExitStack

    f32 = mybir.dt.float32
    u32 = mybir.dt.uint32
    X = mybir.AxisListType.X
    ADD = mybir.AluOpType.add
    MULT = mybir.AluOpType.mult
    COPY = mybir.ActivationFunctionType.Copy

    nc = bacc.Bacc(
        "TRN2", target_bir_lowering=False, debug=False, num_devices=NCORES
    )
    x = nc.declare_dram_parameter("x", [BL, F, D], f32, isOutput=False)
    w1bc = nc.declare_dram_parameter("w1bc", [128, R * F], f32, isOutput=False)
    w2bc = nc.declare_dram_parameter("w2bc", [128, F * R], f32, isOutput=False)
    b1b = nc.declare_dram_parameter("b1b", [128, R], f32, isOutput=False)
    b2b = nc.declare_dram_parameter("b2b", [128, F], f32, isOutput=False)
    offc = nc.declare_dram_parameter("offc", [128, GROUPS], f32, isOutput=False)
    out = nc.declare_dram_parameter("out", [BL, K, D], f32, isOutput=True)

    x_ap = x[:]
    x_flat = x_ap.rearrange("b f d -> (b f) d")  # [BL*64, 512]
    out_ap = out[:]

    def bcast_mid(ap, n):
        # [P, A] -> [P, n, A] with a stride-0 broadcast middle dim
        return bass.AP(
            tensor=ap.tensor, offset=ap.offset, ap=[ap.ap[0], [0, n], *ap.ap[1:]]
        )

    with ExitStack() as ctx:
        tc = ctx.enter_context(tile.TileContext(nc))
        consts = ctx.enter_context(tc.tile_pool(name="consts", bufs=1))
        xpool = ctx.enter_context(tc.tile_pool(name="xpool", bufs=2))
        small = ctx.enter_context(tc.tile_pool(name="small", bufs=2))
        gpool = ctx.enter_context(tc.tile_pool(name="gpool", bufs=2))

        w1bc_sb = consts.tile([128, R * F], f32)
        nc.gpsimd.dma_start(out=w1bc_sb[:], in_=w1bc[:])
        w2bc_sb = consts.tile([128, F * R], f32)
        nc.gpsimd.dma_start(out=w2bc_sb[:], in_=w2bc[:])
        b1b_sb = consts.tile([128, R], f32)
        nc.gpsimd.dma_start(out=b1b_sb[:], in_=b1b[:])
        b2b_sb = consts.tile([128, F], f32)
        nc.gpsimd.dma_start(out=b2b_sb[:], in_=b2b[:])
        offc_sb = consts.tile([128, GROUPS], f32)
        nc.gpsimd.dma_start(out=offc_sb[:], in_=offc[:])
        dummy = consts.tile([128, D], f32)  # ACT elementwise out sink

        for g in range(GROUPS):
            s0 = g * 128
            pooled = small.tile([128, F], f32, tag="pooled")
            for i in range(NLOAD):
                xt = xpool.tile([128, FPL, D], f32, tag="xt")
                nc.sync.dma_start(
                    out=xt[:],
                    in_=x_ap[s0 : s0 + 128, i * FPL : (i + 1) * FPL, :],
                )
                if i == 0:
                    # ---- ACT frame sums (sequential-512 accumulate) ----
                    for fl in range(FPL):
                        nc.scalar.activation(
                            out=dummy[:],
                            in_=xt[:, fl, :],
                            func=COPY,
                            accum_out=pooled[:, fl : fl + 1],
                        )
                else:
                    # Tile 1 is split: DVE two-stage tree on frames 32..51,
                    # ACT sequential sums on 52..63 - both run right after
                    # the load (shorter last-group drain tail), and the
                    # 44/20 frame split keeps both engines ~25% under the
                    # DMA roofline (ACT ~142us, DVE ~73us per call).
                    NDVE = 20
                    part = small.tile([128, NDVE, 16], f32, tag="part")
                    nc.vector.tensor_reduce(
                        out=part[:],
                        in_=xt[:, 0:NDVE, :].rearrange(
                            "p c (s t) -> p c s t", t=32
                        ),
                        axis=X,
                        op=ADD,
                    )
                    nc.vector.tensor_reduce(
                        out=pooled[:, FPL : FPL + NDVE],
                        in_=part[:],
                        axis=X,
                        op=ADD,
                    )
                    for fl in range(NDVE, FPL):
                        nc.scalar.activation(
                            out=dummy[:],
                            in_=xt[:, fl, :],
                            func=COPY,
                            accum_out=pooled[:, FPL + fl : FPL + fl + 1],
                        )

            # ---- SE-MLP, entirely on DVE ----
            # h[p, r] = sum_f pooled[p, f] * w1bc[p, r*64+f]  (w1bc pre-scaled)
            tmp1 = small.tile([128, R, F], f32, tag="tmp1")
            nc.vector.tensor_tensor(
                out=tmp1[:],
                in0=bcast_mid(pooled[:], R),
                in1=w1bc_sb[:].rearrange("p (r f) -> p r f", f=F),
                op=MULT,
            )
            hr = small.tile([128, R], f32, tag="hr")
            nc.vector.tensor_reduce(out=hr[:], in_=tmp1[:], axis=X, op=ADD)
            nc.vector.tensor_add(out=hr[:], in0=hr[:], in1=b1b_sb[:])
            nc.vector.tensor_scalar_max(hr[:], hr[:], 0.0)

            # logits[p, f] = sum_r h[p, r] * w2bc[p, f*16+r]
            tmp2 = small.tile([128, F, R], f32, tag="tmp2")
            nc.vector.tensor_tensor(
                out=tmp2[:],
                in0=bcast_mid(hr[:], F),
                in1=w2bc_sb[:].rearrange("p (f r) -> p f r", r=R),
                op=MULT,
            )
            lg = small.tile([128, F], f32, tag="lg")
            nc.vector.tensor_reduce(out=lg[:], in_=tmp2[:], axis=X, op=ADD)
            nc.vector.tensor_add(out=lg[:], in0=lg[:], in1=b2b_sb[:])

            # ---- top-8 ----
            mx8 = small.tile([128, K], f32, tag="mx8")
            idx8 = small.tile([128, K], u32, tag="idx8")
            nc.vector.max(out=mx8[:], in_=lg[:])
            nc.vector.max_index(out=idx8[:], in_max=mx8[:], in_values=lg[:])

            # global x_flat row id = (g*128 + p)*64 + idx; f32 math is exact
            # for values < 2^24 and tensor_scalar AP operands must be f32
            idx8f = small.tile([128, K], f32, tag="idx8f")
            nc.vector.tensor_copy(out=idx8f[:], in_=idx8[:])
            nc.vector.tensor_scalar(
                out=idx8f[:],
                in0=idx8f[:],
                scalar1=offc_sb[:, g : g + 1],
                scalar2=None,
                op0=ADD,
            )
            grow = small.tile([128, K], u32, tag="grow")
            nc.vector.tensor_copy(out=grow[:], in_=idx8f[:])

            # ---- gather + store ----
            gt = gpool.tile([128, K, D], f32, tag="gt")
            for k in range(K):
                nc.gpsimd.indirect_dma_start(
                    out=gt[:, k, :],
                    out_offset=None,
                    in_=x_flat,
                    in_offset=bass.IndirectOffsetOnAxis(ap=grow[:, k : k + 1], axis=0),
                )
            nc.gpsimd.dma_start(out=out_ap[s0 : s0 + 128, :, :], in_=gt[:])

    nc.compile()
    return nc


def _consts(w1, b1, w2, b2):
    w1s = (np.asarray(w1, np.float32) / 512.0).reshape(1, -1)  # [1, 16*64], r-major
    w2f = np.asarray(w2, np.float32).reshape(1, -1)  # [1, 64*16], f-major
    w1bc = np.ascontiguousarray(np.tile(w1s, (128, 1)))
    w2bc = np.ascontiguousarray(np.tile(w2f, (128, 1)))
    b1b = np.tile(np.asarray(b1, np.float32)[None, :], (128, 1))
    b2b = np.tile(np.asarray(b2, np.float32)[None, :], (128, 1))
    p = np.arange(128)
    offc = (
        (p[:, None] + np.arange(GROUPS)[None, :] * 128) * 64
    ).astype(np.float32)
    return w1bc, w2bc, b1b, b2b, offc


def make_in_maps(x, w1, b1, w2, b2):
    x = np.asarray(x)
    w1bc, w2bc, b1b, b2b, offc = _consts(
        np.asarray(w1), np.asarray(b1), np.asarray(w2), np.asarray(b2)
    )
    in_maps = []
    for i in range(NCORES):
        in_maps.append(
            {
                "x": np.ascontiguousarray(x[i * BL : (i + 1) * BL]),
                "w1bc": w1bc,
                "w2bc": w2bc,
                "b1b": b1b,
                "b2b": b2b,
                "offc": offc,
            }
        )
    return in_maps


def build_nc():
    if "nc" not in _cache:
        _cache["nc"] = _build_nc()
    return _cache["nc"]


def _assemble(by_name):
    return by_name["out"].reshape(NCORES * BL, K, D)


def _fast_call(in_maps):
    """Cached jit of the 8-core NEFF (same construction run_bass_kernel_spmd
    lowers to under axon); repeat kernel() calls skip re-tracing/re-jitting."""
    import jax
    from jax.sharding import Mesh, PartitionSpec
    from jax.experimental.shard_map import shard_map
    from concourse import bass2jax, mybir

    if "fast" not in _cache:
        nc = build_nc()
        bass2jax.install_neuronx_cc_hook()
        partition_name = (
            nc.partition_id_tensor.name if nc.partition_id_tensor else None
        )
        in_names, out_names, out_avals = [], [], []
        for alloc in nc.m.functions[0].allocations:
            if not isinstance(alloc, mybir.MemoryLocationSet):
                continue
            name = alloc.memorylocations[0].name
            if alloc.kind == "ExternalInput":
                if name != partition_name:
                    in_names.append(name)
            elif alloc.kind == "ExternalOutput":
                out_names.append(name)
                out_avals.append(
                    jax.core.ShapedArray(
                        tuple(alloc.tensor_shape), mybir.dt.np(alloc.dtype)
                    )
                )
        all_in = list(in_names) + list(out_names)
        if partition_name:
            all_in.append(partition_name)

        def _body(*args):
            operands = list(args)
            if partition_name:
                operands.append(bass2jax.partition_id_tensor())
            return tuple(
                bass2jax._bass_exec_p.bind(
                    *operands,
                    out_avals=tuple(out_avals),
                    in_names=tuple(all_in),
                    out_names=tuple(out_names),
                    lowering_input_output_aliases=(),
                    sim_require_finite=True,
                    sim_require_nnan=True,
                    nc=nc,
                )
            )

        mesh = Mesh(np.asarray(jax.devices()[:NCORES]), ("core",))
        nin = len(in_names) + len(out_names)
        f = jax.jit(
            shard_map(
                _body,
                mesh=mesh,
                in_specs=(PartitionSpec("core"),) * nin,
                out_specs=(PartitionSpec("core"),) * len(out_names),
                check_rep=False,
            ),
            keep_unused=True,
        )
        _cache["fast"] = (f, in_names, out_names, out_avals)
    f, in_names, out_names, out_avals = _cache["fast"]
    concat_in = [
        np.concatenate([np.asarray(m[nm]) for m in in_maps], axis=0)
        for nm in in_names
    ]
    concat_zero = [
        np.zeros((NCORES * a.shape[0], *a.shape[1:]), a.dtype) for a in out_avals
    ]
    outs = f(*concat_in, *concat_zero)
    oi = out_names.index("out")
    return np.asarray(outs[oi]).reshape(NCORES * BL, K, D)


def kernel(x, w1, b1, w2, b2):
    import os

    # the NTFF trace hook (antenv.axon_hooks) doesn't exist in this container;
    # make sure an inherited BASS_TRACE can't route us onto that path
    os.environ["BASS_NEVER_TRACE"] = "1"
    in_maps = make_in_maps(x, w1, b1, w2, b2)
    try:
        return _fast_call(in_maps)
    except Exception:
        from concourse.bass_utils import run_bass_kernel_spmd

        res = run_bass_kernel_spmd(build_nc(), in_maps, list(range(NCORES)))
        return np.concatenate([r["out"] for r in res.results], axis=0)
